# revision 44
# baseline (speedup 1.0000x reference)
"""Multi-head attention (B=2, S=4096, D=512, H=8) on 8 trn2 NeuronCores.

Sharding (ARCH C): 2 heads x half-q per core. Core c = (head-pair c//2,
q-half c%2): it computes heads {2hp, 2hp+1} for q-positions
[qh*2048, (qh+1)*2048) of BOTH batches, against the full K/V (trimmed to
vlp = ceil(vl/128)*128). Each core applies its 128-row slice of Wo on
device; the host sums the 4 head-pair partials per q-half (the
tensor-parallel all-reduce, done in the gather step). Halving q per core
halves the q projection and the in/out DMA vs 1-head-per-core; stacking
two heads makes the Wo contraction 128-deep.

Per-core dataflow (fp16 matmuls, PSUM f32; see build_kernel_c for the
detailed stage comments): per k-chunk group, 2 score matmuls (one per
head, disjoint partition row-halves) -> ONE exp activation over
[128, 2, 512] (both heads; ACT is the critical engine ~102us busy) ->
8 FLIPPED AV matmuls ou[q, head, 65] += e_slice^T @ vtilde (output
free-size 65 instead of 512: PE matmul cost is output-rows only, so the
flip halves AV's PE time; vtilde's ones-column accumulates the softmax
denominator directly in q-partition layout). PSUM zero-region rule: one
start/stop per 2KB bank per accumulation lifetime. The Wo tail
(normalize -> transpose-via-identity-matmul -> Wo -> f16 copy) is cut
into per-q-sub pieces drained one per group so PE bursts never starve
ACT; projections likewise emit in half-blocks paced by the group-pop
feeder with deadlines from a dry-run of the stream schedule.
"""

import math
import os
from contextlib import ExitStack

import ml_dtypes
import numpy as np

import concourse.bass as bass
import concourse.mybir as mybir
import concourse.tile as tile
from concourse import bacc
from concourse import bass_utils

F32 = mybir.dt.float32
F16 = mybir.dt.float16
EXP = mybir.ActivationFunctionType.Exp
NEG = -1.0e6

N_CORES = 8

# Problem shape (hardcoded per harness contract).
B_, S_, D_, H_ = 2, 4096, 512, 8
HD_ = D_ // H_


def _ceil_div(a, b):
    return (a + b - 1) // b


def _blocks(total, width):
    out = []
    off = 0
    while off < total:
        out.append((off, min(width, total - off)))
        off += width
    return out


def build_kernel(nc, cfg):
    """Emit the per-core kernel IR. cfg keys: S, D, HD, vls (actual valid
    lens per batch), repeat."""
    S, D, HD = cfg["S"], cfg["D"], cfg["HD"]
    mdt = F16
    vls = cfg["vls"]
    B = len(vls)
    vlps = [min(S, _ceil_div(max(v, 1), 128) * 128) for v in vls]
    ND = D // 128
    scale = 1.0 / math.sqrt(HD)
    nch = [v // 128 for v in vlps]
    # chunks that need the NEG bias column (vl not 128-aligned)
    bnd = [vls[b] % 128 != 0 for b in range(B)]
    QB = 512
    assert S % QB == 0
    nqb = S // QB
    NQ = QB // 128  # 128-chunks per q-block

    # ---- DRAM I/O ----
    qT = nc.dram_tensor("qT", [B, D, S], mdt, kind="ExternalInput").ap()
    kTs = [
        nc.dram_tensor(f"kT{b}", [D, vlps[b]], mdt, kind="ExternalInput").ap()
        for b in range(B)
    ]
    vTs = [
        nc.dram_tensor(f"vT{b}", [D, vlps[b]], mdt, kind="ExternalInput").ap()
        for b in range(B)
    ]
    wqd = nc.dram_tensor("wqd", [D, 128], mdt, kind="ExternalInput").ap()
    wkd = nc.dram_tensor("wkd", [D, 128], mdt, kind="ExternalInput").ap()
    wv = nc.dram_tensor("wv", [D, HD], mdt, kind="ExternalInput").ap()
    wo = nc.dram_tensor("wo", [HD, D], mdt, kind="ExternalInput").ap()
    maskb = nc.dram_tensor("maskb", [128, B], F32, kind="ExternalInput").ap()
    out = nc.dram_tensor("out", [B, S, D], F16, kind="ExternalOutput").ap()

    with tile.TileContext(nc) as tc, ExitStack() as ctx:
        consts = ctx.enter_context(tc.tile_pool(name="consts", bufs=1))
        xt = ctx.enter_context(tc.tile_pool(name="xt", bufs=cfg.get("bufs_xt", 14)))
        qkv = ctx.enter_context(tc.tile_pool(name="qkv", bufs=1))
        epool = ctx.enter_context(tc.tile_pool(name="e", bufs=cfg.get("bufs_e", 4)))
        ousb = ctx.enter_context(tc.tile_pool(name="ousb", bufs=3))
        stage = ctx.enter_context(tc.tile_pool(name="stage", bufs=3))
        small = ctx.enter_context(tc.tile_pool(name="small", bufs=2))
        ps_mm = ctx.enter_context(
            tc.tile_pool(name="ps_mm", bufs=cfg.get("bufs_mm", 2), space="PSUM")
        )
        ps_sc = ctx.enter_context(
            tc.tile_pool(name="ps_sc", bufs=cfg.get("bufs_sc", 2), space="PSUM")
        )
        ps_ou = ctx.enter_context(tc.tile_pool(name="ps_ou", bufs=2, space="PSUM"))

        # ---- constants (tiles only; DMAs are emitted by load_consts after
        # the first data loads so the first q/k tiles aren't queued behind
        # the weight transfers) ----
        wqd_sb = consts.tile([128, ND, 128], mdt)
        wkd_sb = consts.tile([128, ND, 128], mdt)
        wv_sb = consts.tile([128, ND, HD], mdt)
        wo_sb = consts.tile([HD, D], mdt)
        maskb_sb = consts.tile([128, B], F32)
        # unit2: [HD+1, 2] with row HD ones; extracts the denominator row of
        # outU as a [q, 2] column pair via one tiny matmul per q-chunk.
        unit2_f32 = consts.tile([HD + 1, 2], F32)
        nc.vector.memset(unit2_f32, 0.0)
        nc.vector.memset(unit2_f32[HD : HD + 1, :], 1.0)
        unit2 = consts.tile([HD + 1, 2], mdt)
        nc.vector.tensor_copy(unit2, unit2_f32)
        # warmup operand: the PE clock unthrottles only after ~3.4us of
        # sustained activity, so burn idle fill time on dummy matmuls.
        warm = consts.tile([64, QB], mdt)
        nc.vector.memset(warm, 0.0)

        max_nch = max(nch)
        ones_stage = consts.tile([128, max_nch, 1], F32)
        nc.vector.memset(ones_stage, 1.0)

        consts_loaded = [False]
        warmed = [False]

        def load_consts():
            if consts_loaded[0]:
                return
            consts_loaded[0] = True
            for w_sb, w_ap in ((wqd_sb, wqd), (wkd_sb, wkd)):
                nc.sync.dma_start(
                    out=w_sb, in_=w_ap.rearrange("(c p) h -> p c h", p=128)
                )
            nc.sync.dma_start(out=wv_sb, in_=wv.rearrange("(c p) h -> p c h", p=128))
            nc.sync.dma_start(out=wo_sb, in_=wo)
            nc.sync.dma_start(out=maskb_sb, in_=maskb)

        def emit():
            # persistent projected tensors, per batch (dup row-halves)
            qT_sb = [
                qkv.tile([128, S], mdt, name=f"qT_sb{b}") for b in range(B)
            ]
            kT_sb = [
                qkv.tile([128, vlps[b]], mdt, name=f"kT_sb{b}") for b in range(B)
            ]
            vbuf = [
                qkv.tile([128, nch[b], HD + 1], mdt, name=f"vbuf{b}")
                for b in range(B)
            ]
            for b in range(B):
                nc.vector.tensor_copy(
                    vbuf[b][:, :, HD : HD + 1], ones_stage[:, : nch[b], :]
                )

            def load_xt(src_ap, soff, swidth):
                # One DMA per 512-block: [128, ND, sw] (partition = row within
                # d-chunk). Merging the per-chunk loads matters: each DMA
                # instruction costs ~625ns of serialized HWDGE descriptor-gen.
                t = xt.tile([128, ND, QB], mdt, tag="xt")
                nc.sync.dma_start(
                    out=t[:, :, :swidth],
                    in_=src_ap.rearrange("(c p) s -> p c s", p=128)[
                        :, :, soff : soff + swidth
                    ],
                )
                return t

            # ---- projection matmul pieces (DMA split out so loads can be
            # issued far ahead of the PE FIFO reaching the matmuls) ----
            def mm_k(b, soff, sw, t):
                ps = ps_mm.tile([128, QB], F32, tag="mm")
                for dc in range(ND):
                    nc.tensor.matmul(
                        ps[:, :sw],
                        wkd_sb[:, dc, :],
                        t[:, dc, :sw],
                        start=(dc == 0),
                        stop=(dc == ND - 1),
                    )
                nc.vector.tensor_copy(kT_sb[b][:, soff : soff + sw], ps[:, :sw])

            def mm_v(b, soff, sw, t):
                for sub in range(sw // 128):
                    ps = ps_mm.tile([128, HD], F32, tag="mm")
                    for dc in range(ND):
                        nc.tensor.matmul(
                            ps,
                            t[:, dc, sub * 128 : (sub + 1) * 128],
                            wv_sb[:, dc, :],
                            start=(dc == 0),
                            stop=(dc == ND - 1),
                        )
                    kc = (soff + sub * 128) // 128
                    nc.vector.tensor_copy(vbuf[b][:, kc, 0:HD], ps)

            def mm_q(b, soff, sw, t):
                ps = ps_mm.tile([128, QB], F32, tag="mm")
                for dc in range(ND):
                    nc.tensor.matmul(
                        ps[:, :sw],
                        wqd_sb[:, dc, :],
                        t[:, dc, :sw],
                        start=(dc == 0),
                        stop=(dc == ND - 1),
                    )
                nc.vector.tensor_copy(qT_sb[b][:, soff : soff + sw], ps[:, :sw])

            def proj_q_block(b, soff):
                t = load_xt(qT[b], soff, QB)
                mm_q(b, soff, QB, t)

            # ---- Wo + normalize + out DMA for one finished q-block.
            # mixed=True sends half the normalize-copies to the (then idle)
            # ACT engine to break the serial wps->normalize chain; used when
            # the tail is emitted into a short k-loop or at the very end.
            # split_dma=True overlaps the out DMA with the chain (final tail).
            def tail_rest(b, qb, ou_sb, mixed=False, split_dma=False):
                qoff = qb * QB
                recip = small.tile([128, NQ], F32, tag="recip")
                st = stage.tile([128, NQ, D], F16)
                for qi in range(NQ):
                    dps = ps_mm.tile([128, 2], F32, tag="mm")
                    nc.tensor.matmul(
                        dps,
                        ou_sb[0 : HD + 1, qi * 128 : (qi + 1) * 128],
                        unit2,
                        start=True,
                        stop=True,
                    )
                    nc.vector.reciprocal(recip[:, qi : qi + 1], dps[:, 0:1])
                    wps = ps_mm.tile([128, D], F32, tag="mm")
                    nc.tensor.matmul(
                        wps,
                        ou_sb[0:HD, qi * 128 : (qi + 1) * 128],
                        wo_sb,
                        start=True,
                        stop=True,
                    )
                    if mixed and qi % 2:
                        nc.scalar.activation(
                            st[:, qi, :],
                            wps,
                            mybir.ActivationFunctionType.Copy,
                            scale=recip[:, qi : qi + 1],
                        )
                    else:
                        nc.vector.tensor_scalar_mul(
                            st[:, qi, :], wps, recip[:, qi : qi + 1]
                        )
                    if split_dma:
                        nc.sync.dma_start(
                            out=out[
                                b, qoff + qi * 128 : qoff + (qi + 1) * 128, :
                            ].rearrange("(q p) n -> p q n", p=128),
                            in_=st[:, qi : qi + 1, :],
                        )
                if not split_dma:
                    nc.sync.dma_start(
                        out=out[b, qoff : qoff + QB, :].rearrange(
                            "(q p) n -> p q n", p=128
                        ),
                        in_=st,
                    )

            # ---- one attention "stream" = the k-loop of one q-block,
            # steppable one chunk-group at a time so TWO streams can be
            # round-robined: while one stream crosses its q-block boundary
            # (tail, psum swap), the other keeps the ACT engine fed. ----
            def make_stream(skey):
                b, qb = skey
                n = nch[b]
                n_int = n - 1 if bnd[b] else n  # interior (maskless) chunks
                qoff = qb * QB
                if cfg.get("flipT"):
                    ou = ps_ou.tile(
                        [128, NQ, HD + 1], F32, name=f"ou_{b}_{qb}", tag="ou"
                    )
                else:
                    ou = ps_ou.tile([HD + 1, QB], F32, name=f"ou_{b}_{qb}", tag="ou")
                st_ = {"key": skey, "kc": 0, "ou": ou}

                def av(kc, e_ap, first, last):
                    if cfg.get("flipT"):
                        for qi in range(NQ):
                            nc.tensor.matmul(
                                ou[:, qi, :],
                                e_ap[:, qi * 128 : (qi + 1) * 128],
                                vbuf[b][:, kc, :],
                                start=first,
                                stop=last,
                            )
                        return
                    nc.tensor.matmul(
                        ou, vbuf[b][:, kc, :], e_ap, start=first, stop=last
                    )

                def step():
                    kc = st_["kc"]
                    if kc < n_int - 1:  # fused interior pair
                        pp = ps_sc.tile([128, 2, QB], F32, tag="pp")
                        for i, half in enumerate((kc, kc + 1)):
                            r0 = 0 if cfg.get("no_rowtile") else i * 64
                            nc.tensor.matmul(
                                pp[:, i, :],
                                kT_sb[b][r0 : r0 + 64, half * 128 : half * 128 + 128],
                                qT_sb[b][r0 : r0 + 64, qoff : qoff + QB],
                                start=True,
                                stop=True,
                            )
                        e = epool.tile([128, 2, QB], mdt, tag="e")
                        nc.scalar.activation(e, pp, EXP, scale=scale)
                        av(kc, e[:, 0, :], kc == 0, False)
                        av(kc + 1, e[:, 1, :], False, kc + 2 == n)
                        st_["kc"] = kc + 2
                    else:  # leftover interior single or boundary chunk
                        pp = ps_sc.tile([128, 2, QB], F32, tag="pp")
                        nc.tensor.matmul(
                            pp[:, 0, :],
                            kT_sb[b][0:64, kc * 128 : kc * 128 + 128],
                            qT_sb[b][0:64, qoff : qoff + QB],
                            start=True,
                            stop=True,
                        )
                        e = epool.tile([128, 2, QB], mdt, tag="e")
                        if kc >= n_int:
                            nc.scalar.activation(
                                e[:, 0, :],
                                pp[:, 0, :],
                                EXP,
                                bias=maskb_sb[:, b : b + 1],
                                scale=scale,
                            )
                        else:
                            nc.scalar.activation(
                                e[:, 0, :], pp[:, 0, :], EXP, scale=scale
                            )
                        av(kc, e[:, 0, :], kc == 0, kc + 1 == n)
                        st_["kc"] = kc + 1
                    return st_["kc"] >= n

                st_["step"] = step
                return st_

            # Stream order: batch-1's short streams interleave between
            # batch-0's long ones (their ACT deficit absorbs into the paired
            # long stream); end on a long stream so the final tails hide.
            MAX_ACTIVE = cfg.get("max_active", 1)
            if MAX_ACTIVE == 1:
                streams = [(0, 0)]
                for j in range(1, nqb - 1):
                    streams += [(0, j), (1, j - 1)]
                streams += [(1, nqb - 2), (1, nqb - 1), (0, nqb - 1)]
            else:
                streams = [(0, 0), (0, 1)]
                for j in range(2, nqb - 1):
                    streams += [(1, j - 2), (0, j)]
                streams += [
                    (1, nqb - 3),
                    (1, nqb - 2),
                    (0, nqb - 1),
                    (1, nqb - 1),
                ]

            def groups_of(b):
                ni = nch[b] - 1 if bnd[b] else nch[b]
                return ni // 2 + ni % 2 + (1 if bnd[b] else 0)

            EARLY = cfg.get("early", 3)

            def drive(mk, after_group, on_finish):
                """Stream driver over `streams`: sequential when MAX_ACTIVE=1
                (with the next stream starting EARLY groups before the
                current one ends, so the ACT engine never sees a q-block
                boundary), full 2-active round-robin when MAX_ACTIVE=2. Used
                with counter stubs for the dry run (feeder deadlines) and
                with real emitters for the actual pass — keeping both
                aligned."""
                idx = 0
                active = []

                def left(s):
                    return groups_of(s["key"][0]) - s.get("_g", 0)

                def add_one():
                    nonlocal idx
                    if idx < len(streams):
                        active.append(mk(streams[idx]))
                        idx += 1

                add_one()
                if MAX_ACTIVE >= 2:
                    add_one()
                while active:
                    for st_ in list(active):
                        if st_ not in active:
                            continue
                        done = st_["step"]()
                        st_["_g"] = st_.get("_g", 0) + 1
                        after_group()
                        if done:
                            active.remove(st_)
                            on_finish(st_)
                        if not active or (
                            len(active) < 2
                            and sum(left(s) for s in active) <= EARLY
                        ):
                            add_one()

            # dry run: group-pop count at which each stream starts
            starts = {}
            dry = {"pops": 0}

            def _mk_dry(skey):
                starts[skey] = dry["pops"]
                st_ = {"key": skey, "g": 0}
                g_total = groups_of(skey[0])

                def step():
                    st_["g"] += 1
                    return st_["g"] >= g_total

                st_["step"] = step
                return st_

            drive(_mk_dry, lambda: dry.update(pops=dry["pops"] + 1), lambda s: None)

            # ---- schedule: the projection work list. Each item is a
            # (dma, mm) pair; DMAs are issued LOOKAHEAD items ahead of their
            # matmuls so the PE FIFO never head-of-line blocks on a load.
            # Ordered by when attention needs the data: q-block 0 and batch-0
            # k/v first, batch-1 entirely under batch-0's attention. ----
            # Pace the items across the group-pops so the PE never gets a
            # bunched run of projection matmuls (which starves ACT).
            # Deadline = the group-pop count after which the item may be
            # emitted; the first four items are primed before the drive.
            kv0_blocks = _blocks(vlps[0], QB)
            n_kv0 = 2 * len(kv0_blocks)
            n_kv1 = 2 * len(_blocks(vlps[1], QB))
            items = []  # (deadline, dma, mm)

            def it(dl, dma, mm):
                items.append((dl, dma, mm))

            it(-1, lambda: load_xt(qT[0], 0, QB), lambda t: mm_q(0, 0, QB, t))
            soff0, sw0 = kv0_blocks[0]
            it(
                -1,
                lambda: load_xt(kTs[0], soff0, sw0),
                lambda t: mm_k(0, soff0, sw0, t),
            )
            it(
                -1,
                lambda: load_xt(vTs[0], soff0, sw0),
                lambda t: mm_v(0, soff0, sw0, t),
            )
            it(-1, lambda: load_xt(qT[0], QB, QB), lambda t: mm_q(0, QB, QB, t))
            for r, (soff, sw) in enumerate(kv0_blocks[1:]):
                it(
                    2 * r,
                    lambda soff=soff, sw=sw: load_xt(kTs[0], soff, sw),
                    lambda t, soff=soff, sw=sw: mm_k(0, soff, sw, t),
                )
                it(
                    2 * r + 1,
                    lambda soff=soff, sw=sw: load_xt(vTs[0], soff, sw),
                    lambda t, soff=soff, sw=sw: mm_v(0, soff, sw, t),
                )
            qslack = cfg.get("qslack", 8)
            for qb in range(2, nqb):
                it(
                    max(n_kv0 - 2, starts[(0, qb)] - qslack),
                    lambda qb=qb: load_xt(qT[0], qb * QB, QB),
                    lambda t, qb=qb: mm_q(0, qb * QB, QB, t),
                )
            for r, (soff, sw) in enumerate(_blocks(vlps[1], QB)):
                dl = max(n_kv0 - 2, starts[(1, 0)] - 2 * (n_kv1 - 2 * r))
                it(
                    dl,
                    lambda soff=soff, sw=sw: load_xt(kTs[1], soff, sw),
                    lambda t, soff=soff, sw=sw: mm_k(1, soff, sw, t),
                )
                it(
                    dl + 1,
                    lambda soff=soff, sw=sw: load_xt(vTs[1], soff, sw),
                    lambda t, soff=soff, sw=sw: mm_v(1, soff, sw, t),
                )
            for qb in range(nqb):
                it(
                    max(n_kv0 - 2, starts[(1, qb)] - qslack),
                    lambda qb=qb: load_xt(qT[1], qb * QB, QB),
                    lambda t, qb=qb: mm_q(1, qb * QB, QB, t),
                )
            sched = sorted(
                ((dl, i, dma, mm) for i, (dl, dma, mm) in enumerate(items)),
                key=lambda x: (x[0], x[1]),
            )

            LOOKAHEAD = cfg.get("lookahead", 6)
            state = {"nxt": 0, "dma": 0, "tiles": {}, "pops": 0}

            def emit_next():
                while state["dma"] < min(len(sched), state["nxt"] + 1 + LOOKAHEAD):
                    j = state["dma"]
                    state["tiles"][j] = sched[j][2]()
                    state["dma"] += 1
                i = state["nxt"]
                sched[i][3](state["tiles"].pop(i))
                state["nxt"] = i + 1

            def pop_feeder():
                p = state["pops"]
                state["pops"] += 1
                if state["nxt"] < len(sched) and sched[state["nxt"]][0] <= p:
                    emit_next()

            # prime: the first two data loads jump the DMA queue ahead of the
            # weight transfers; PE warms its clock on dummy matmuls while
            # they land; then q-blocks 0/1 and the first k/v block project.
            for j in range(2):
                state["tiles"][j] = sched[j][2]()
                state["dma"] = j + 1
            load_consts()
            if not warmed[0]:
                warmed[0] = True
                wps_warm = ps_sc.tile([128, 2, QB], F32, tag="pp")
                for _ in range(7):
                    nc.tensor.matmul(
                        wps_warm[:, 0, :], warm[:, 0:128], warm, start=True, stop=True
                    )
            for _ in range(4):
                emit_next()

            # real pass: round-robin the streams; when one finishes, copy its
            # accumulator out of PSUM immediately (frees the bank for the
            # next stream) and defer the Wo tail a few groups so it lands
            # under the other stream's ACT work.
            tails = []  # [age, closure]

            tail_age = cfg.get("tail_age", 3)

            def after_group():
                pop_feeder()
                for t_ in tails:
                    t_[0] += 1
                if tails and tails[0][0] >= tail_age:
                    tails.pop(0)[1]()

            # timing-only flip experiment: tail consumes a dummy SBUF tile of
            # the old layout (values 1.0) so the instruction mix is unchanged
            # while AV runs in the flipped orientation.
            dummy_ousb = None
            if cfg.get("flipT"):
                dummy_ousb = consts.tile([HD + 1, QB], mdt)
                nc.vector.memset(dummy_ousb, 1.0)

            def on_finish(st_):
                b, qb = st_["key"]
                if cfg.get("flipT"):
                    last = st_["key"] == streams[-1]
                    tails.append(
                        [
                            0,
                            lambda b=b, qb=qb, lt=last: tail_rest(
                                b, qb, dummy_ousb, mixed=lt, split_dma=lt
                            ),
                        ]
                    )
                    return
                ou_sb = ousb.tile([HD + 1, QB], mdt, name=f"ousb_{b}_{qb}", tag="ousb")
                nc.vector.tensor_copy(ou_sb, st_["ou"])
                last = st_["key"] == streams[-1]
                tails.append(
                    [
                        0,
                        lambda b=b, qb=qb, o=ou_sb, lt=last: tail_rest(
                            b, qb, o, mixed=lt, split_dma=lt
                        ),
                    ]
                )

            drive(make_stream, after_group, on_finish)
            while state["nxt"] < len(sched):
                emit_next()
            for _age, fn in tails:
                fn()
            tails.clear()

        for _ in range(cfg.get("repeat", 1)):
            emit()

    nc.compile()
    return nc


def _half_range(sw, half):
    """Column range for a half-block projection (half=None -> full)."""
    if half is None:
        return 0, sw
    h = ((sw // 128) + 1) // 2 * 128 if sw > 128 else sw
    h = min(h, sw)
    return (0, h) if half == 0 else (h, sw)


def _half_subs(nsub, half):
    h = (nsub + 1) // 2
    return range(0, h) if half == 0 else range(h, nsub)


def build_kernel_c(nc, cfg):
    """ARCH C: 2 heads x half-q per core, flipped AV orientation.

    Core c (host side): hp = c//2 owns heads {2hp, 2hp+1}; qh = c%2 owns
    q-positions [qh*S/2, (qh+1)*S/2) of BOTH batches. The kernel is
    SPMD-identical; all per-core differences live in the input slices.

    Per-core dataflow (fp16 matmuls, PSUM f32):
      - projections: wq/wk/wv hold the core's TWO head columns (128 wide,
        no duplication); kT_sb/qT_sb land as [128(2 heads x 64), pos].
      - attention: per q-block, stream k one 128-chunk at a time. One
        chunk-group = 2 score matmuls (one per head, disjoint partition
        row-halves) into pp[128, 2, QB], ONE exp activation over the whole
        [128, 2, QB] tile (both heads, free=1024 amortizes ACT overhead;
        the boundary chunk adds the NEG bias column - valid for both heads
        since the mask depends only on k-position), then 8 flipped AV
        matmuls: ou[qi][128q, h, qi%2, 0:65] += e_chunk_qslice^T @ vtilde.
        Flipped AV outputs 65 free rows/matmul instead of 512 (PE cost is
        output-free-size x cycle, contraction-independent), and column 64
        (ones in vtilde) accumulates the softmax denominator directly in
        q-partition layout.
      - tail per q-block: reciprocal + normalize o (DVE, [128,64] tiles,
        pre-Wo so the big post-Wo tile needs no scale), PE-transpose both
        heads' o into one stacked oT[128(2x64), 128q] (gpsimd copies it
        out of PSUM), ONE Wo matmul per q-sub with contraction 128 (both
        heads at once), final PSUM->f16 copy, DMA out [QB, 512] f16.
    Host gathers: out[b, qh-half] = sum over the 4 head-pair cores.
    """
    S, D, HD = cfg["S"], cfg["D"], cfg["HD"]
    SQ = S // 2  # per-core q positions per batch
    mdt = F16
    vls = cfg["vls"]
    B = len(vls)
    assert B == 2
    vlps = [min(S, _ceil_div(max(v, 1), 128) * 128) for v in vls]
    ND = D // 128
    scale = 1.0 / math.sqrt(HD)
    nch = [v // 128 for v in vlps]
    bnd = [vls[b] % 128 != 0 for b in range(B)]
    QB = 512
    assert SQ % QB == 0
    nqb = SQ // QB  # 4
    NQ = QB // 128  # 4
    H2 = 2 * HD  # 128: two heads stacked on partitions

    # ---- DRAM I/O ----
    qT = nc.dram_tensor("qT", [B, D, SQ], mdt, kind="ExternalInput").ap()
    kTs = [
        nc.dram_tensor(f"kT{b}", [D, vlps[b]], mdt, kind="ExternalInput").ap()
        for b in range(B)
    ]
    vTs = [
        nc.dram_tensor(f"vT{b}", [D, vlps[b]], mdt, kind="ExternalInput").ap()
        for b in range(B)
    ]
    # wq|wk|wv (each host-rearranged to [128, 4, 128]) and wo packed into a
    # single [128, 16, 128] blob: one DMA instead of five (each dma_start
    # costs ~625ns of serialized HWDGE descriptor-gen at startup).
    # weights split into two blobs: wqk gates the first scores (loaded
    # first, 0.73us), wvo+maskb aren't needed until the first AV / boundary
    # exp, so their DMAs queue AFTER the q0/k0 data loads.
    wqk = nc.dram_tensor("wqk", [128, 8, 128], mdt, kind="ExternalInput").ap()
    wvo = nc.dram_tensor("wvo", [128, 8, 128], mdt, kind="ExternalInput").ap()
    maskb = nc.dram_tensor("maskb", [128, B], F32, kind="ExternalInput").ap()
    out = nc.dram_tensor("out", [B, SQ, D], F16, kind="ExternalOutput").ap()

    with tile.TileContext(nc) as tc, ExitStack() as ctx:
        consts = ctx.enter_context(tc.tile_pool(name="consts", bufs=1))
        xt = ctx.enter_context(tc.tile_pool(name="xt", bufs=cfg.get("bufs_xt", 14)))
        qkv = ctx.enter_context(
            tc.tile_pool(name="qkv", bufs=cfg.get("bufs_qkv", 2))
        )
        epool = ctx.enter_context(tc.tile_pool(name="e", bufs=cfg.get("bufs_e", 7)))
        osb = ctx.enter_context(tc.tile_pool(name="osb", bufs=cfg.get("bufs_osb", 3)))
        otb = ctx.enter_context(tc.tile_pool(name="otb", bufs=cfg.get("bufs_otb", 3)))
        stage = ctx.enter_context(tc.tile_pool(name="stage", bufs=cfg.get("bufs_stage", 3)))
        small = ctx.enter_context(tc.tile_pool(name="small", bufs=2))
        ps_mm = ctx.enter_context(
            tc.tile_pool(name="ps_mm", bufs=cfg.get("bufs_mm", 2), space="PSUM")
        )
        ps_sc = ctx.enter_context(
            tc.tile_pool(name="ps_sc", bufs=cfg.get("bufs_sc", 2), space="PSUM")
        )
        ps_ou = ctx.enter_context(tc.tile_pool(name="ps_ou", bufs=2, space="PSUM"))

        # ---- constants ----
        wqk_sb = consts.tile([128, 8, 128], mdt)
        wvo_sb = consts.tile([128, 8, 128], mdt)
        wq_sb = wqk_sb[:, 0:4, :]
        wk_sb = wqk_sb[:, 4:8, :]
        wv_sb = wvo_sb[:, 0:4, :]
        wo_sb = wvo_sb[:, 4:8, :]
        maskb_sb = consts.tile([128, B], F32)
        # identity for PE transposes
        ident = consts.tile([128, 128], mdt)
        nc.vector.memset(ident, 1.0)
        nc.gpsimd.affine_select(
            ident,
            ident,
            pattern=[[1, 128]],
            compare_op=mybir.AluOpType.is_equal,
            fill=0.0,
            base=0,
            channel_multiplier=-1,
        )
        # PE clock warmup operand
        warm = consts.tile([64, QB], mdt)
        nc.vector.memset(warm, 0.0)

        consts_loaded = [False]
        wvo_loaded = [False]
        warmed = [False]

        def load_consts():
            if consts_loaded[0]:
                return
            consts_loaded[0] = True
            nc.sync.dma_start(out=wqk_sb, in_=wqk)

        def load_wvo():
            if wvo_loaded[0]:
                return None
            wvo_loaded[0] = True
            nc.sync.dma_start(out=wvo_sb, in_=wvo)
            nc.sync.dma_start(out=maskb_sb, in_=maskb)
            return None

        def emit():
            qT_sb = [qkv.tile([128, SQ], mdt, name=f"qT_sb{b}") for b in range(B)]
            kT_sb = [
                qkv.tile([128, vlps[b]], mdt, name=f"kT_sb{b}") for b in range(B)
            ]
            # vtilde: [128 kpos, chunk, head, 65]; col 64 = ones (denominator)
            vbuf = [
                qkv.tile([128, nch[b], 2, HD + 1], mdt, name=f"vbuf{b}")
                for b in range(B)
            ]
            for b in range(B):
                nc.vector.memset(vbuf[b][:, :, :, HD : HD + 1], 1.0)

            def load_xt(src_ap, soff, swidth):
                t = xt.tile([128, ND, QB], mdt, tag="xt")
                nc.sync.dma_start(
                    out=t[:, :, :swidth],
                    in_=src_ap.rearrange("(c p) s -> p c s", p=128)[
                        :, :, soff : soff + swidth
                    ],
                )
                return t

            # projections emit in HALF-blocks (half the output columns) so
            # each PE burst (~430ns) fits the per-group PE slack under ACT.
            def mm_k(b, soff, sw, t, half=None):
                o0, o1 = _half_range(sw, half)
                if o1 <= o0:
                    return
                ps = ps_mm.tile([128, QB // 2], F32, tag="mm")
                for dc in range(ND):
                    nc.tensor.matmul(
                        ps[:, : o1 - o0],
                        wk_sb[:, dc, :],
                        t[:, dc, o0:o1],
                        start=(dc == 0),
                        stop=(dc == ND - 1),
                    )
                nc.vector.tensor_copy(
                    kT_sb[b][:, soff + o0 : soff + o1], ps[:, : o1 - o0]
                )

            def mm_v(b, soff, sw, t, half=None):
                nsub = sw // 128
                subs = range(nsub) if half is None else _half_subs(nsub, half)
                for sub in subs:
                    ps = ps_mm.tile([128, 2, HD], F32, tag="mm")
                    for dc in range(ND):
                        nc.tensor.matmul(
                            ps,
                            t[:, dc, sub * 128 : (sub + 1) * 128],
                            wv_sb[:, dc, :],
                            start=(dc == 0),
                            stop=(dc == ND - 1),
                        )
                    kc = (soff + sub * 128) // 128
                    nc.vector.tensor_copy(vbuf[b][:, kc, :, 0:HD], ps)

            def mm_q(b, soff, sw, t, half=None):
                o0, o1 = _half_range(sw, half)
                if o1 <= o0:
                    return
                ps = ps_mm.tile([128, QB // 2], F32, tag="mm")
                for dc in range(ND):
                    nc.tensor.matmul(
                        ps[:, : o1 - o0],
                        wq_sb[:, dc, :],
                        t[:, dc, o0:o1],
                        start=(dc == 0),
                        stop=(dc == ND - 1),
                    )
                nc.vector.tensor_copy(
                    qT_sb[b][:, soff + o0 : soff + o1], ps[:, : o1 - o0]
                )

            # ---- Wo tail piece for ONE q-sub of a finished q-block
            # (consumes the normalized o_sb). Emitted one piece per group so
            # the PE burst (~320ns) fits inside the per-group PE slack and
            # never starves ACT. last=True routes the final copy to the
            # then-idle ACT engine and DMAs per q-sub.
            def tail_piece(b, qb, o_sb, st, qi, last=False):
                qoff = qb * QB
                # transpose both heads' o via REGULAR matmul with identity
                # rhs (out = o^T @ I) - is_transpose mode costs a PE
                # pipeline flush on hw that the plain matmul avoids.
                tr = ps_mm.tile([128, 128], F32, tag="mm")
                for h in range(2):
                    # disjoint partition ranges: each keeps its own
                    # start/stop (zero regions are per-partition)
                    nc.tensor.matmul(
                        tr[64 * h : 64 * h + 64, :],
                        o_sb[:, qi, h, :],
                        ident,
                        start=True,
                        stop=True,
                    )
                oT = otb.tile([128, 128], mdt, tag="oT")
                nc.vector.tensor_copy(oT, tr)
                wps = ps_mm.tile([128, D], F32, tag="mm")
                nc.tensor.matmul(wps, oT, wo_sb, start=True, stop=True)
                if last and qi % 2 == 0:
                    nc.scalar.activation(
                        st[:, qi, :], wps, mybir.ActivationFunctionType.Copy
                    )
                else:
                    nc.vector.tensor_copy(st[:, qi, :], wps)
                if last:
                    nc.sync.dma_start(
                        out=out[
                            b, qoff + qi * 128 : qoff + (qi + 1) * 128, :
                        ].rearrange("(q p) n -> p q n", p=128),
                        in_=st[:, qi : qi + 1, :],
                    )
                elif qi == NQ - 1:
                    nc.sync.dma_start(
                        out=out[b, qoff : qoff + QB, :].rearrange(
                            "(q p) n -> p q n", p=128
                        ),
                        in_=st,
                    )

            # ---- one attention stream = k-loop of one q-block, one chunk
            # per step (scores both heads -> exp -> 8 flipped AV matmuls).
            def make_stream(skey):
                b, qb = skey
                n = nch[b]
                qoff = qb * QB
                # [128q, head, qi%2, 128]: each accumulator region padded to
                # 512B so the four matmul accumulation groups in the bank
                # stay in disjoint aligned windows (unaligned packing lets a
                # start=True zero a neighbour region's window).
                ou = [
                    ps_ou.tile(
                        [128, 2, 2, 128],
                        F32,
                        name=f"ou_{b}_{qb}_{i}",
                        tag="ou",
                    )
                    for i in range(2)
                ]
                st_ = {"key": skey, "kc": 0, "ou": ou}

                def step():
                    kc = st_["kc"]
                    pp = ps_sc.tile([128, 2, QB], F32, tag="pp")
                    for h in range(2):
                        nc.tensor.matmul(
                            pp[:, h, :],
                            kT_sb[b][64 * h : 64 * h + 64, kc * 128 : kc * 128 + 128],
                            qT_sb[b][64 * h : 64 * h + 64, qoff : qoff + QB],
                            start=True,
                            stop=True,
                        )
                    e = epool.tile([128, 2, QB], mdt, tag="e")
                    if kc == n - 1 and bnd[b]:
                        nc.scalar.activation(
                            e, pp, EXP, bias=maskb_sb[:, b : b + 1], scale=scale
                        )
                    else:
                        nc.scalar.activation(e, pp, EXP, scale=scale)
                    # PSUM zero-region semantics: start=True marks the WHOLE
                    # 2KB bank pending-zero, so each ou bank gets exactly one
                    # start (its first matmul, kc==0) and one stop (its last);
                    # the other regions initialize via pending-zero writes.
                    first, last_ = kc == 0, kc + 1 == n
                    for h in range(2):
                        for qi in range(NQ):
                            nc.tensor.matmul(
                                ou[qi // 2][:, h, qi % 2, 0 : HD + 1],
                                e[:, h, qi * 128 : (qi + 1) * 128],
                                vbuf[b][:, kc, h, :],
                                start=first and h == 0 and qi % 2 == 0,
                                stop=last_ and h == 1 and qi % 2 == 1,
                            )
                    st_["kc"] = kc + 1
                    return st_["kc"] >= n

                st_["step"] = step
                return st_

            # Stream order: long batch L's streams carry the schedule; the
            # short batch's streams slot between them; end on a long stream
            # so the final tails hide under its k-loop.
            L = 0 if nch[0] >= nch[1] else 1
            Sh = 1 - L
            assert nqb == 4
            streams = [
                (L, 0),
                (L, 1),
                (Sh, 0),
                (L, 2),
                (Sh, 1),
                (Sh, 2),
                (Sh, 3),
                (L, 3),
            ]

            def groups_of(b):
                return nch[b]

            EARLY = cfg.get("early", 1)

            def drive(mk, after_group, on_finish):
                idx = 0
                active = []

                def left(s):
                    return groups_of(s["key"][0]) - s.get("_g", 0)

                def add_one():
                    nonlocal idx
                    if idx < len(streams):
                        active.append(mk(streams[idx]))
                        idx += 1

                add_one()
                while active:
                    for st_ in list(active):
                        if st_ not in active:
                            continue
                        done = st_["step"]()
                        st_["_g"] = st_.get("_g", 0) + 1
                        after_group()
                        if done:
                            active.remove(st_)
                            on_finish(st_)
                        if not active or (
                            len(active) < 2
                            and sum(left(s) for s in active) <= EARLY
                        ):
                            add_one()

            # dry run: group-pop count at which each stream starts
            starts = {}
            dry = {"pops": 0}

            def _mk_dry(skey):
                starts[skey] = dry["pops"]
                st_ = {"key": skey, "g": 0}
                g_total = groups_of(skey[0])

                def step():
                    st_["g"] += 1
                    return st_["g"] >= g_total

                st_["step"] = step
                return st_

            drive(_mk_dry, lambda: dry.update(pops=dry["pops"] + 1), lambda s: None)

            # ---- projection schedule, paced by deadlines over group-pops.
            # Each logical block becomes TWO half-items sharing one DMA.
            kvL_blocks = _blocks(vlps[L], QB)
            kvS_blocks = _blocks(vlps[Sh], QB)
            n_kvS = 2 * len(kvS_blocks)
            items = []

            def it(dl, dma, mm):
                items.append((dl, dma, mm))

            def it2(dl, src_ap, soff, sw, mm_fn, b):
                cell = {}

                def dma_a():
                    cell["t"] = load_xt(src_ap, soff, sw)
                    return cell["t"]

                it(dl, dma_a, lambda t: mm_fn(b, soff, sw, t, 0))
                it(dl, lambda: cell["t"], lambda t: mm_fn(b, soff, sw, t, 1))

            it2(-1, qT[L], 0, QB, mm_q, L)
            soff0, sw0 = kvL_blocks[0]
            it2(-1, kTs[L], soff0, sw0, mm_k, L)
            it(-1, load_wvo, lambda t: None)
            it2(-1, vTs[L], soff0, sw0, mm_v, L)
            it2(-1, qT[L], QB, QB, mm_q, L)
            # each 512-block of k/v feeds 4 chunk-groups; consumption is one
            # chunk per group, so block r must land by group ~4r.
            for r, (soff, sw) in enumerate(kvL_blocks[1:], start=1):
                it2(4 * r - 4, kTs[L], soff, sw, mm_k, L)
                it2(4 * r - 2, vTs[L], soff, sw, mm_v, L)
            qslack = cfg.get("qslack", 6)
            n_kvL = 2 * len(kvL_blocks)
            for qb in range(2, nqb):
                it2(
                    max(n_kvL - 2, starts[(L, qb)] - qslack),
                    qT[L],
                    qb * QB,
                    QB,
                    mm_q,
                    L,
                )
            for r, (soff, sw) in enumerate(kvS_blocks):
                dl = max(n_kvL - 2, starts[(Sh, 0)] - 2 * (n_kvS - 2 * r))
                it2(dl, kTs[Sh], soff, sw, mm_k, Sh)
                it2(dl + 2, vTs[Sh], soff, sw, mm_v, Sh)
            for qb in range(nqb):
                it2(
                    max(n_kvL - 2, starts[(Sh, qb)] - qslack),
                    qT[Sh],
                    qb * QB,
                    QB,
                    mm_q,
                    Sh,
                )
            sched = sorted(
                ((dl, i, dma, mm) for i, (dl, dma, mm) in enumerate(items)),
                key=lambda x: (x[0], x[1]),
            )

            LOOKAHEAD = cfg.get("lookahead", 6)
            state = {"nxt": 0, "dma": 0, "tiles": {}, "pops": 0}

            def emit_next():
                while state["dma"] < min(len(sched), state["nxt"] + 1 + LOOKAHEAD):
                    j = state["dma"]
                    state["tiles"][j] = sched[j][2]()
                    state["dma"] += 1
                i = state["nxt"]
                sched[i][3](state["tiles"].pop(i))
                state["nxt"] = i + 1

            def pop_feeder():
                p = state["pops"]
                state["pops"] += 1
                if state["nxt"] < len(sched) and sched[state["nxt"]][0] <= p:
                    emit_next()

            load_consts()
            for j in range(4):
                state["tiles"][j] = sched[j][2]()
                state["dma"] = j + 1
            if not warmed[0]:
                warmed[0] = True
                wps_warm = ps_sc.tile([128, 2, QB], F32, tag="pp")
                for _ in range(7):
                    nc.tensor.matmul(
                        wps_warm[:, 0, :], warm[:, 0:128], warm, start=True, stop=True
                    )
            for _ in range(8):
                emit_next()

            tails = []  # [age, closure]
            tail_age = cfg.get("tail_age", 8)

            def after_group():
                pop_feeder()
                for t_ in tails:
                    t_[0] += 1
                drained = 0
                while tails and tails[0][0] >= tail_age and drained < (
                    2 if len(tails) > cfg.get("drain_hi", 8) else 1
                ):
                    tails.pop(0)[1]()
                    drained += 1

            def tail_dummy(b, qb):
                qoff = qb * QB
                st = stage.tile([128, NQ, D], F16)
                for qi in range(NQ):
                    nc.vector.tensor_copy(st[:, qi, 0:128], ident)
                nc.sync.dma_start(
                    out=out[b, qoff : qoff + QB, :].rearrange(
                        "(q p) n -> p q n", p=128
                    ),
                    in_=st,
                )

            def on_finish(st_):
                b, qb = st_["key"]
                if cfg.get("dummy_tail"):
                    tails.append([0, lambda b=b, qb=qb: tail_dummy(b, qb)])
                    return
                # prompt: reciprocal + normalize (frees the ou PSUM banks).
                # One strided reciprocal per ou tile covers its 4 denominators.
                rc = small.tile([128, 2, 2, 2], F32, tag="rc")
                o_sb = osb.tile([128, NQ, 2, HD], mdt, tag="osb")
                for ti in range(2):
                    nc.vector.reciprocal(
                        rc[:, ti, :, :], st_["ou"][ti][:, :, :, HD : HD + 1]
                    )
                last = st_["key"] == streams[-1]
                for qi in range(NQ):
                    for h in range(2):
                        if last and (qi * 2 + h) % 2:
                            # end-of-kernel: no exp work left, so ACT takes
                            # half the normalizes to shorten the tail chain
                            nc.scalar.activation(
                                o_sb[:, qi, h, :],
                                st_["ou"][qi // 2][:, h, qi % 2, 0:HD],
                                mybir.ActivationFunctionType.Copy,
                                scale=rc[:, qi // 2, h, qi % 2 : qi % 2 + 1],
                            )
                        else:
                            nc.vector.tensor_scalar_mul(
                                o_sb[:, qi, h, :],
                                st_["ou"][qi // 2][:, h, qi % 2, 0:HD],
                                rc[:, qi // 2, h, qi % 2 : qi % 2 + 1],
                            )
                st = stage.tile([128, NQ, D], F16)
                for qi in range(NQ):
                    tails.append(
                        [
                            0,
                            lambda b=b, qb=qb, o=o_sb, s=st, qi=qi, lt=last: (
                                tail_piece(b, qb, o, s, qi, lt)
                            ),
                        ]
                    )

            drive(make_stream, after_group, on_finish)
            while state["nxt"] < len(sched):
                emit_next()
            for _age, fn in tails:
                fn()
            tails.clear()

        for _ in range(cfg.get("repeat", 1)):
            emit()

    nc.compile()
    return nc


def prepare_in_maps_c(queries, keys, values, vls, Wq, Wk, Wv, Wo, vlps,
                      np_dt=np.float16):
    """Host-side prep for ARCH C: core c = (head-pair c//2, q-half c%2)."""
    HD = HD_
    SQ = S_ // 2
    queries, keys, values = (x.astype(np_dt) for x in (queries, keys, values))
    Wq, Wk, Wv, Wo = (x.astype(np_dt) for x in (Wq, Wk, Wv, Wo))
    qT_full = queries.transpose(0, 2, 1)  # [B, D, S]
    kT = [np.ascontiguousarray(keys[b].T[:, : vlps[b]]) for b in range(B_)]
    vT = [np.ascontiguousarray(values[b].T[:, : vlps[b]]) for b in range(B_)]
    nch = [v // 128 for v in vlps]
    maskb_np = np.zeros((128, B_), dtype=np.float32)
    for b in range(B_):
        pos = (nch[b] - 1) * 128 + np.arange(128)
        maskb_np[:, b] = np.where(pos < vls[b], 0.0, NEG)

    qT_half = [
        np.ascontiguousarray(qT_full[:, :, qh * SQ : (qh + 1) * SQ])
        for qh in range(2)
    ]

    def _rearr(w):  # [512, 128] -> [128, 4, 128] (p c h layout)
        return w.reshape(4, 128, 128).transpose(1, 0, 2)

    in_maps = []
    for c in range(N_CORES):
        hp, qh = c // 2, c % 2
        h0 = hp * 2 * HD
        wqk = np.concatenate(
            [_rearr(Wq[:, h0 : h0 + 2 * HD]), _rearr(Wk[:, h0 : h0 + 2 * HD])],
            axis=1,
        )
        wvo = np.concatenate(
            [
                _rearr(Wv[:, h0 : h0 + 2 * HD]),
                Wo[h0 : h0 + 2 * HD, :].reshape(128, 4, 128),
            ],
            axis=1,
        )
        m = {
            "qT": qT_half[qh],
            "wqk": np.ascontiguousarray(wqk),
            "wvo": np.ascontiguousarray(wvo),
            "maskb": maskb_np,
        }
        for b in range(B_):
            m[f"kT{b}"] = kT[b]
            m[f"vT{b}"] = vT[b]
        in_maps.append(m)
    return in_maps


def prepare_in_maps(queries, keys, values, vls, Wq, Wk, Wv, Wo, vlps,
                    np_dt=np.float16):
    """Host-side layout prep: transposes, trims, per-core weight slices, mask."""
    HD = HD_
    queries, keys, values = (x.astype(np_dt) for x in (queries, keys, values))
    Wq, Wk, Wv, Wo = (x.astype(np_dt) for x in (Wq, Wk, Wv, Wo))
    qT = np.ascontiguousarray(queries.transpose(0, 2, 1))          # [B, D, S]
    kT = [np.ascontiguousarray(keys[b].T[:, : vlps[b]]) for b in range(B_)]
    vT = [np.ascontiguousarray(values[b].T[:, : vlps[b]]) for b in range(B_)]
    nch = [v // 128 for v in vlps]
    maskb_np = np.zeros((128, B_), dtype=np.float32)
    for b in range(B_):
        pos = (nch[b] - 1) * 128 + np.arange(128)
        maskb_np[:, b] = np.where(pos < vls[b], 0.0, NEG)

    in_maps = []
    for c in range(N_CORES):
        h0 = c * HD
        wq_h = Wq[:, h0 : h0 + HD]
        wk_h = Wk[:, h0 : h0 + HD]
        m = {
            "qT": qT,
            "wqd": np.ascontiguousarray(np.concatenate([wq_h, wq_h], axis=1)),
            "wkd": np.ascontiguousarray(np.concatenate([wk_h, wk_h], axis=1)),
            "wv": np.ascontiguousarray(Wv[:, h0 : h0 + HD]),
            "wo": np.ascontiguousarray(Wo[h0 : h0 + HD, :]),
            "maskb": maskb_np,
        }
        for b in range(B_):
            m[f"kT{b}"] = kT[b]
            m[f"vT{b}"] = vT[b]
        in_maps.append(m)
    return in_maps


_NC_CACHE = {}

DEFAULT_DT = "f16"


def _get_nc(cfg_key):
    if cfg_key not in _NC_CACHE:
        S, D, HD, vls = cfg_key
        nc = bacc.Bacc(
            "TRN2",
            target_bir_lowering=False,
            debug=False,
            enable_asserts=False,
            num_devices=N_CORES,
        )
        build_kernel_c(nc, {"S": S, "D": D, "HD": HD, "vls": vls})
        _NC_CACHE[cfg_key] = nc
    return _NC_CACHE[cfg_key]


LAST_RESULT = None  # BassKernelResults of the most recent kernel() call
LAST_IN_MAPS = None


def kernel(queries, keys, values, valid_lens, Wq, Wk, Wv, Wo, _trace=False):
    global LAST_RESULT, LAST_IN_MAPS
    queries = np.ascontiguousarray(np.asarray(queries, dtype=np.float32))
    keys = np.ascontiguousarray(np.asarray(keys, dtype=np.float32))
    values = np.ascontiguousarray(np.asarray(values, dtype=np.float32))
    Wq = np.ascontiguousarray(np.asarray(Wq, dtype=np.float32))
    Wk = np.ascontiguousarray(np.asarray(Wk, dtype=np.float32))
    Wv = np.ascontiguousarray(np.asarray(Wv, dtype=np.float32))
    Wo = np.ascontiguousarray(np.asarray(Wo, dtype=np.float32))
    vls = tuple(int(v) for v in np.asarray(valid_lens).reshape(-1))

    Bq, S, D = queries.shape
    assert (Bq, S, D) == (B_, S_, D_), (Bq, S, D)
    HD = HD_
    vlps = tuple(min(S, _ceil_div(max(v, 1), 128) * 128) for v in vls)

    nc = _get_nc((S, D, HD, vls))
    in_maps = prepare_in_maps_c(
        queries, keys, values, vls, Wq, Wk, Wv, Wo, vlps, np_dt=np.float16
    )
    LAST_IN_MAPS = in_maps
    LAST_RESULT = bass_utils.run_bass_kernel_spmd(
        nc, in_maps, core_ids=list(range(N_CORES)), trace=_trace
    )
    SQ = S // 2
    acc = np.zeros((B_, S, D), dtype=np.float32)
    for c, r in enumerate(LAST_RESULT.results):
        qh = c % 2
        acc[:, qh * SQ : (qh + 1) * SQ, :] += r["out"]
    return acc

